# revision 10
# baseline (speedup 1.0000x reference)
"""Trainium2 Bass kernel for the 3DSSG edge-classification GNN.

Model (see reference): per-node embeddings (orientation lookup + dimension
linear), per-edge gather of (subj, rel, obj) features, 3-layer MLP over
E=1M candidate edges producing [E, 27] scores.

Strategy
--------
All per-node linear maps are folded into the first MLP layer, so each node is
described by 30 raw features: [dimension(3), location(3), onehot24(orient)].
Layer 1 becomes  h1 = relu(u[src] @ P + u[dst] @ Q + c)  with P,Q [30->128pad, 256].

Edges are data-parallel across 8 cores. Node features live in SBUF as fp16
tables (rows padded to 128 fp16 = 256B, the dma_gather element size); per-edge
features are fetched with transposed SBUF-source dma_gather (feature-major
output, directly consumable as the matmul moving operand). dma_gather indices
are int16, so the node table is split into two halves of 25088 rows; edges are
bucketed on the host by (src-half, dst-half) into 4 classes, each class is
split across 2 cores, and every core runs the identical SPMD program against
its own half-tables + index streams. Outputs are written as [27, E_core] and
inverse-permuted on the host.

Precision: fp16 features/weights with hi/lo compensation slots on layer-1
(exact-ish layer 1); fp32 PSUM accumulation everywhere; h1/h2 in fp16.
"""

import sys

import numpy as np

import concourse.bacc as bacc
import concourse.mybir as mybir
from concourse.tile import TileContext
from concourse import library_config
from concourse.bass_utils import run_bass_kernel_spmd

# ---------------------------------------------------------------- constants
N = 50000
SPLIT = 25000          # node-half boundary
HALF = 25088           # padded half rows (196 ranks x 128)
RANKS = HALF // 128
G = 2048               # edges per gather group
SUB = 512              # edges per matmul subtile
H1 = 256
H2 = 128
C1 = 27                # output classes
F16 = mybir.dt.float16
F32 = mybir.dt.float32
I16 = mybir.dt.int16

_PROGRAM_CACHE = {}


# ---------------------------------------------------------------- host prep
def _fold(dimension, location, orientation, orient_table, size_W, size_b,
          subj_W, subj_b, obj_W, obj_b, trans_W, trans_b, mlp_W1, mlp_b1):
    """Fold node-embedding linears into layer-1 weights.

    Returns fp16 node feature tables (two halves, [HALF,128]), fp16 lhsT
    weight blocks P1,P2,Q1,Q2 [128,128], and fp32 bias c [256].
    """
    f = np.float32
    W1a, W1b, W1c = mlp_W1[:128].astype(f), mlp_W1[128:256].astype(f), mlp_W1[256:].astype(f)
    A1 = subj_W.astype(f) @ W1a            # [64, 256]
    A2 = obj_W.astype(f) @ W1c             # [64, 256]
    B1 = trans_W.astype(f) @ W1b           # [3, 256]

    # raw per-node blocks: continuous x = [dimension(3), location(3)],
    # onehot o = onehot24(orientation)
    Pc = np.concatenate([size_W.astype(f) @ A1[:32], B1], 0)      # [6, 256]
    Qc = np.concatenate([size_W.astype(f) @ A2[:32], -B1], 0)     # [6, 256]
    Po = orient_table.astype(f) @ A1[32:]                          # [24, 256]
    Qo = orient_table.astype(f) @ A2[32:]                          # [24, 256]
    c = (mlp_b1.astype(f)
         + size_b.astype(f) @ A1[:32] + subj_b.astype(f) @ W1a
         + size_b.astype(f) @ A2[:32] + obj_b.astype(f) @ W1c
         + trans_b.astype(f) @ W1b)                                # [256]

    x = np.concatenate([dimension.astype(f), location.astype(f)], 1)  # [N, 6]
    x_hi = x.astype(np.float16)
    x_lo = (x - x_hi.astype(f)).astype(np.float16)
    oh = np.zeros((N, 24), np.float16)
    oh[np.arange(N), orientation] = 1.0

    # feature slot blocks: (table columns fp16 [N,k], weight rows fp32 [k,256])
    Pc_hi = Pc.astype(np.float16).astype(f)
    Po_hi = Po.astype(np.float16).astype(f)
    Qc_hi = Qc.astype(np.float16).astype(f)
    Qo_hi = Qo.astype(np.float16).astype(f)
    blocks = [
        (x_hi, Pc, Qc),              # hi features x hi weights (fp16 rounds rows)
        (oh, Po, Qo),
        (x_lo, Pc_hi, Qc_hi),        # lo features x hi weights
        (x_hi, Pc - Pc_hi, Qc - Qc_hi),  # hi features x weight residual
        (oh, Po - Po_hi, Qo - Qo_hi),
    ]
    cols = []
    prows = []
    qrows = []
    for tcols, pr, qr in blocks:
        cols.append(tcols)
        prows.append(pr)
        qrows.append(qr)
    U = np.concatenate(cols, 1)                    # [N, 66]
    P = np.concatenate(prows, 0)                   # [66, 256]
    Q = np.concatenate(qrows, 0)

    nslots = U.shape[1]
    assert nslots <= 128
    U128 = np.zeros((N, 128), np.float16)
    U128[:, :nslots] = U.astype(np.float16)
    P128 = np.zeros((128, 256), f)
    P128[:nslots] = P
    Q128 = np.zeros((128, 256), f)
    Q128[:nslots] = Q

    # tables in SBUF layout: node n -> partition n%128, rank n//128.
    # host pre-transposes to [128, RANKS*128] so the device load is one
    # contiguous-per-partition 2D DMA (128 big descriptors, not 25088).
    def to_sbuf_layout(rows):
        t = np.zeros((HALF, 128), np.float16)
        t[:len(rows)] = rows
        return t.reshape(RANKS, 128, 128).transpose(1, 0, 2).reshape(128, RANKS * 128)

    t0 = to_sbuf_layout(U128[:SPLIT])
    t1 = to_sbuf_layout(U128[SPLIT:])
    P128 = P128.astype(np.float16)
    Q128 = Q128.astype(np.float16)
    return (t0, t1,
            P128[:, :128], P128[:, 128:], Q128[:, :128], Q128[:, 128:], c)


def _idx_layout(idx, e_pc):
    """int16 gather layout: idx i at [i%16, i//16], replicated to 128 parts."""
    pad = np.empty(e_pc, np.int16)
    pad[:len(idx)] = idx
    if len(idx) < e_pc:
        pad[len(idx):] = idx[0] if len(idx) else 0
    lay = pad.reshape(e_pc // 16, 16).T.copy()     # [16, e_pc/16]
    return np.tile(lay, (8, 1))                     # [128, e_pc/16]


# ---------------------------------------------------------------- program
def _build(n_groups):
    e_pc = n_groups * G
    nc = bacc.Bacc("TRN2", target_bir_lowering=False)
    tsrc_d = nc.dram_tensor("tsrc", [128, RANKS * 128], F16, kind="ExternalInput")
    tdst_d = nc.dram_tensor("tdst", [128, RANKS * 128], F16, kind="ExternalInput")
    isrc_d = nc.dram_tensor("isrc", [128, e_pc // 16], I16, kind="ExternalInput")
    idst_d = nc.dram_tensor("idst", [128, e_pc // 16], I16, kind="ExternalInput")
    wp1_d = nc.dram_tensor("wp1", [128, 128], F16, kind="ExternalInput")
    wp2_d = nc.dram_tensor("wp2", [128, 128], F16, kind="ExternalInput")
    wq1_d = nc.dram_tensor("wq1", [128, 128], F16, kind="ExternalInput")
    wq2_d = nc.dram_tensor("wq2", [128, 128], F16, kind="ExternalInput")
    w2a_d = nc.dram_tensor("w2a", [128, 128], F16, kind="ExternalInput")
    w2b_d = nc.dram_tensor("w2b", [128, 128], F16, kind="ExternalInput")
    w3_d = nc.dram_tensor("w3", [128, C1], F16, kind="ExternalInput")
    ca_d = nc.dram_tensor("ca", [128, 1], F32, kind="ExternalInput")
    cb_d = nc.dram_tensor("cb", [128, 1], F32, kind="ExternalInput")
    b2_d = nc.dram_tensor("b2", [128, 1], F32, kind="ExternalInput")
    b3_d = nc.dram_tensor("b3", [C1, 1], F32, kind="ExternalInput")
    out_d = nc.dram_tensor("scores", [C1, e_pc], F32, kind="ExternalOutput")

    Relu = mybir.ActivationFunctionType.Relu
    add = mybir.AluOpType.add
    amax = mybir.AluOpType.max

    with TileContext(nc) as tc:
        with tc.tile_pool(name="const", bufs=1) as cpool, \
             tc.tile_pool(name="gat", bufs=2) as gpool, \
             tc.tile_pool(name="act", bufs=3) as apool, \
             tc.tile_pool(name="psum", bufs=2, space="PSUM") as ppool:
            nc.gpsimd.load_library(library_config.mlp)

            tsrc_s = cpool.tile([128, RANKS * 128], F16)
            nc.sync.dma_start(out=tsrc_s[:, :], in_=tsrc_d[:, :])
            tdst_s = cpool.tile([128, RANKS * 128], F16)
            nc.sync.dma_start(out=tdst_s[:, :], in_=tdst_d[:, :])
            isrc_s = cpool.tile([128, e_pc // 16], I16)
            nc.sync.dma_start(out=isrc_s[:, :], in_=isrc_d[:, :])
            idst_s = cpool.tile([128, e_pc // 16], I16)
            nc.sync.dma_start(out=idst_s[:, :], in_=idst_d[:, :])

            consts = {}
            for name, dram in (("wp1", wp1_d), ("wp2", wp2_d), ("wq1", wq1_d),
                               ("wq2", wq2_d), ("w2a", w2a_d), ("w2b", w2b_d),
                               ("w3", w3_d), ("ca", ca_d), ("cb", cb_d),
                               ("b2", b2_d), ("b3", b3_d)):
                t = cpool.tile(dram.shape, dram.dtype, name=name)
                nc.sync.dma_start(out=t[:, :], in_=dram[:, :])
                consts[name] = t

            for g in range(n_groups):
                gs = gpool.tile([128, 1, G], F16, name="gs")
                nc.gpsimd.dma_gather(
                    gs[:, :, :], tsrc_s[:, :],
                    isrc_s[:, g * (G // 16):(g + 1) * (G // 16)],
                    G, G, 128, transpose=True, single_packet=False,
                    sbuf_tokens_per_rank=128, sbuf_free_dim_per_rank=256)
                gd = gpool.tile([128, 1, G], F16, name="gd")
                nc.gpsimd.dma_gather(
                    gd[:, :, :], tdst_s[:, :],
                    idst_s[:, g * (G // 16):(g + 1) * (G // 16)],
                    G, G, 128, transpose=True, single_packet=False,
                    sbuf_tokens_per_rank=128, sbuf_free_dim_per_rank=256)

                for t in range(G // SUB):
                    cs = slice(t * SUB, (t + 1) * SUB)
                    ps1a = ppool.tile([128, SUB], F32, name="ps1a")
                    nc.tensor.matmul(ps1a[:, :], consts["wp1"][:, :],
                                     gs[:, 0, cs], start=True, stop=False)
                    nc.tensor.matmul(ps1a[:, :], consts["wq1"][:, :],
                                     gd[:, 0, cs], start=False, stop=True)
                    ps1b = ppool.tile([128, SUB], F32, name="ps1b")
                    nc.tensor.matmul(ps1b[:, :], consts["wp2"][:, :],
                                     gs[:, 0, cs], start=True, stop=False)
                    nc.tensor.matmul(ps1b[:, :], consts["wq2"][:, :],
                                     gd[:, 0, cs], start=False, stop=True)

                    h1a = apool.tile([128, SUB], F16, name="h1a")
                    nc.scalar.activation(h1a[:, :], ps1a[:, :], Relu,
                                         bias=consts["ca"][:, :])
                    h1b = apool.tile([128, SUB], F16, name="h1b")
                    nc.vector.tensor_scalar(h1b[:, :], ps1b[:, :],
                                            consts["cb"][:, :], 0.0, add, amax)

                    ps2 = ppool.tile([128, SUB], F32, name="ps2")
                    nc.tensor.matmul(ps2[:, :], consts["w2a"][:, :],
                                     h1a[:, :], start=True, stop=False)
                    nc.tensor.matmul(ps2[:, :], consts["w2b"][:, :],
                                     h1b[:, :], start=False, stop=True)
                    h2 = apool.tile([128, SUB], F16, name="h2")
                    nc.scalar.activation(h2[:, :], ps2[:, :], Relu,
                                         bias=consts["b2"][:, :])

                    ps3 = ppool.tile([C1, SUB], F32, name="ps3")
                    nc.tensor.matmul(ps3[:, :], consts["w3"][:, :],
                                     h2[:, :], start=True, stop=True)
                    sc = apool.tile([C1, SUB], F32, name="sc")
                    nc.vector.tensor_scalar(sc[:, :], ps3[:, :],
                                            consts["b3"][:, :], None, add)
                    nc.sync.dma_start(
                        out=out_d[:, g * G + t * SUB: g * G + (t + 1) * SUB],
                        in_=sc[:, :])
    nc.compile()
    return nc


# ---------------------------------------------------------------- driver
def _prepare(inputs):
    src = np.asarray(inputs["src"], np.int64)
    dst = np.asarray(inputs["dst"], np.int64)
    E = src.shape[0]

    t0, t1, P1, P2, Q1, Q2, c = _fold(
        np.asarray(inputs["dimension"]), np.asarray(inputs["location"]),
        np.asarray(inputs["orientation"]), np.asarray(inputs["orient_table"]),
        np.asarray(inputs["size_W"]), np.asarray(inputs["size_b"]),
        np.asarray(inputs["subj_W"]), np.asarray(inputs["subj_b"]),
        np.asarray(inputs["obj_W"]), np.asarray(inputs["obj_b"]),
        np.asarray(inputs["trans_W"]), np.asarray(inputs["trans_b"]),
        np.asarray(inputs["mlp_W1"]), np.asarray(inputs["mlp_b1"]))
    tables = (t0, t1)

    # bucket edges by (src-half, dst-half); each class split across 2 cores
    cls = (src >= SPLIT).astype(np.int64) * 2 + (dst >= SPLIT)
    perm = np.argsort(cls, kind="stable")
    counts = np.bincount(cls, minlength=4)
    slices = []           # (perm_lo, perm_hi, src_half, dst_half) per core
    off = 0
    for k in range(4):
        n = int(counts[k])
        h = (n + 1) // 2
        slices.append((off, off + h, k >> 1, k & 1))
        slices.append((off + h, off + n, k >> 1, k & 1))
        off += n
    max_e = max(b - a for a, b, _, _ in slices)
    n_groups = max(1, -(-max_e // G))
    e_pc = n_groups * G

    in_maps = []
    base = {
        "wp1": P1, "wp2": P2, "wq1": Q1, "wq2": Q2,
        "ca": c[:128].reshape(128, 1).astype(np.float32),
        "cb": c[128:].reshape(128, 1).astype(np.float32),
        "w2a": np.asarray(inputs["mlp_W2"])[:128].astype(np.float16),
        "w2b": np.asarray(inputs["mlp_W2"])[128:].astype(np.float16),
        "b2": np.asarray(inputs["mlp_b2"]).reshape(128, 1).astype(np.float32),
        "w3": np.asarray(inputs["mlp_W3"]).astype(np.float16),
        "b3": np.asarray(inputs["mlp_b3"]).reshape(C1, 1).astype(np.float32),
    }
    for a, b, sh, dh in slices:
        es = src[perm[a:b]] - sh * SPLIT
        ed = dst[perm[a:b]] - dh * SPLIT
        m = dict(base)
        m["tsrc"] = tables[sh]
        m["tdst"] = tables[dh]
        m["isrc"] = _idx_layout(es.astype(np.int16), e_pc)
        m["idst"] = _idx_layout(ed.astype(np.int16), e_pc)
        in_maps.append(m)
    return in_maps, slices, perm, n_groups, E


def _assemble(results, slices, perm, E):
    scores = np.empty((E, C1), np.float32)
    for core, (a, b, _, _) in enumerate(slices):
        n = b - a
        scores[perm[a:b]] = results[core]["scores"][:, :n].T
    return scores


def kernel(**inputs):
    in_maps, slices, perm, n_groups, E = _prepare(inputs)
    if n_groups not in _PROGRAM_CACHE:
        _PROGRAM_CACHE[n_groups] = _build(n_groups)
    nc = _PROGRAM_CACHE[n_groups]
    res = run_bass_kernel_spmd(nc, in_maps, list(range(8)))
    scores = _assemble(res.results, slices, perm, E)
    return scores, np.asarray(inputs["targets"])


def _install_ntff_shim():
    """Register the libaxon NTFF profiling hook if antenv.axon_hooks is
    missing in this image (mirrors trn_agent_boot._ntff_profile_via_ctypes)."""
    import contextlib
    import ctypes
    import sys as _sys
    import types

    try:
        from antenv.axon_hooks import get_axon_ntff_profile_hook  # noqa: F401
        return
    except ImportError:
        pass

    so_path = "/opt/axon/libaxon_pjrt.so"
    lib = ctypes.CDLL(so_path)
    if not hasattr(lib, "axon_start_nrt_profile"):
        return
    lib.axon_start_nrt_profile.argtypes = [
        ctypes.POINTER(ctypes.c_int64), ctypes.c_size_t]
    lib.axon_start_nrt_profile.restype = ctypes.c_int64
    lib.axon_stop_nrt_profile.argtypes = [ctypes.c_char_p]
    lib.axon_stop_nrt_profile.restype = ctypes.c_int64

    @contextlib.contextmanager
    def _hook(output_dir, device_ids):
        import jax
        jax.devices()
        if device_ids:
            ids = (ctypes.c_int64 * len(device_ids))(*device_ids)
            rc = lib.axon_start_nrt_profile(ids, len(device_ids))
        else:
            rc = lib.axon_start_nrt_profile(None, 0)
        if rc != 0:
            raise RuntimeError(f"axon_start_nrt_profile rc={rc}")
        try:
            yield
        finally:
            n = lib.axon_stop_nrt_profile(str(output_dir).encode())
            print(f"ntff profile: {n} file(s) -> {output_dir}", file=sys.stderr)

    holder = {"h": _hook}
    mod = types.ModuleType("antenv.axon_hooks")
    mod.set_axon_ntff_profile_hook = lambda h: holder.__setitem__("h", h)
    mod.get_axon_ntff_profile_hook = lambda: holder.get("h")
    import antenv
    _sys.modules["antenv.axon_hooks"] = mod
    antenv.axon_hooks = mod

    import concourse.bass_utils as _bu
    _bu.upload_artifacts = lambda tmpdir: f"local:{tmpdir}"


def kernel_traced(**inputs):
    """Like kernel() but with NTFF tracing; returns (out, BassKernelResults)."""
    _install_ntff_shim()
    in_maps, slices, perm, n_groups, E = _prepare(inputs)
    if n_groups not in _PROGRAM_CACHE:
        _PROGRAM_CACHE[n_groups] = _build(n_groups)
    nc = _PROGRAM_CACHE[n_groups]
    res = run_bass_kernel_spmd(nc, in_maps, list(range(8)), trace=True)
    scores = _assemble(res.results, slices, perm, E)
    return (scores, np.asarray(inputs["targets"])), res
